# revision 8
# baseline (speedup 1.0000x reference)
"""HMM forward (negative log-marginal) on 8 TRN2 NeuronCores.

Algorithm: the log-space recurrence
    alpha_t[b,j] = obs_t[b,j] + LSE_i(alpha_{t-1}[b,i] + T_log[j,i])
is run in linear space with per-step host-precomputed normalizers:
    aD_t[j,b] = (eobs_t[j,b] / (sigma_tb * SW)) * sum_i Wq[i,j] * aD_{t-1}[i,b]
where Wq = fp8_e4m3(W * SW), sigma_tb = pi_star . eobs_t[:,b] is a rank-1
prediction of the per-step growth (keeps aD in fp8 range; measured drift
is only +-0.2 nats), and the log-scales are summed on the host:
    -log p = -(log sum_j aD_255 + sum_t log sigma_tb + logC0 - 255*SHIFT).

The matmul chain runs in fp8e4m3 DoubleRow perf mode: each instruction
contracts 256 rows (2 fp8 packed per PE cell), so a step is 8 LDW+MM
pairs instead of 16 and the LoadStationary traffic (the baseline
bottleneck: 2048 of 2176 PE cycles/step) is halved.

Sharding: data-parallel over batch (64 -> 8 per core), W replicated.
Device layout [z, batch]: z chunk of 128 on partitions, batch on the
free axis; the DoubleRow pair dim coincides with the psum j-chunk pair
layout, so no transposes are needed anywhere.
"""

import numpy as np
import ml_dtypes

Z = 512
X = 10000
SEQ = 256
B = 64
NCORES = 8
BS = B // NCORES  # 8 batch per core
P = 128
ZC = Z // P  # 4 z-chunks
KC = 2       # DoubleRow contraction chunks (2 x 256)
SHIFT = 9.2
SW = 2048.0      # fp8 W scale (max entry ~203 < 240)
ASC = 1.5        # initial alpha mean (vector max/mean ~92 -> max ~140 < 240)
TCH = 51  # eobs t-chunk (5 * 51 = 255)
NCH = (SEQ - 1) // TCH

_NC_CACHE = {}


def _build_nc():
    if "nc" in _NC_CACHE:
        return _NC_CACHE["nc"]
    from concourse import bacc
    import concourse.mybir as mybir
    import concourse.tile as tile

    bf16 = mybir.dt.bfloat16
    fp8 = mybir.dt.float8e4
    f32 = mybir.dt.float32
    DR = mybir.MatmulPerfMode.DoubleRow

    nc = bacc.Bacc("TRN2", target_bir_lowering=False, debug=False,
                   num_devices=NCORES)

    # w[p, kc, pair, j] = Wq[kc*256 + pair*128 + p, j]
    w_d = nc.dram_tensor("w", [KC, P, 2, Z], fp8, kind="ExternalInput")
    eobs_d = nc.dram_tensor("eobs", [P, SEQ - 1, ZC, BS], bf16,
                            kind="ExternalInput")
    ae0_d = nc.dram_tensor("ae0", [P, ZC, BS], fp8, kind="ExternalInput")
    out_d = nc.dram_tensor("out", [1, BS], f32, kind="ExternalOutput")

    with tile.TileContext(nc) as tc:
        with (
            tc.tile_pool(name="constp", bufs=1) as constp,
            tc.tile_pool(name="aep", bufs=2) as aep,
            tc.tile_pool(name="psp", bufs=2, space="PSUM") as psp,
            tc.tile_pool(name="finp", bufs=1) as finp,
        ):
            w_sb = []
            for kc in range(KC):
                wt = constp.tile([P, 2, Z], fp8, name=f"w_sb{kc}",
                                 tag=f"w_sb{kc}")
                nc.sync.dma_start(out=wt[:], in_=w_d[kc])
                w_sb.append(wt)

            ae_init = constp.tile([P, ZC, BS], fp8, name="ae_init")
            nc.sync.dma_start(out=ae_init[:], in_=ae0_d[:])

            ones_sb = constp.tile([P, 1], fp8, name="ones_sb")
            nc.vector.memset(ones_sb[:], 1.0)
            # Load the Ln table set early so the final log doesn't stall.
            scr_in = finp.tile([P, 1], f32, name="scr_in")
            nc.vector.memset(scr_in[:], 1.0)
            scratch = finp.tile([P, 1], f32, name="scratch")
            nc.scalar.activation(scratch[:], scr_in[:],
                                 mybir.ActivationFunctionType.Ln)

            eobs_sb = []
            for k in range(NCH):
                et = constp.tile([P, TCH, ZC, BS], bf16, name=f"eobs_{k}",
                                 tag=f"eobs_{k}")
                nc.sync.dma_start(out=et[:],
                                  in_=eobs_d[:, k * TCH:(k + 1) * TCH, :, :])
                eobs_sb.append(et)

            # Slot order per step: psA (j 0..255) completes by slot 4 so its
            # DVE evacuation (-> aeA, the kc=0 input) overlaps slots 5-8;
            # aeB (kc=1 input) is first consumed at slot 3 of the next step.
            ORDER = [(0, 0), (1, 0), (0, 1), (1, 1),
                     (2, 0), (3, 0), (2, 1), (3, 1)]
            # prev[kc] = alpha pairs [P, 2, BS] for contraction rows
            # kc*256 .. kc*256+255
            prev = [ae_init[:, 0:2, :], ae_init[:, 2:4, :]]
            for t in range(1, SEQ):
                k, toff = divmod(t - 1, TCH)
                psA = psp.tile([P, 2, 512], f32, tag="psA", name=f"psA_{t}")
                psB = psp.tile([P, 2, 512], f32, tag="psB", name=f"psB_{t}")
                pspair = [psA, psB]
                for (jc, kc) in ORDER:
                    nc.tensor.matmul(
                        pspair[jc // 2][:, jc % 2, 0:BS],
                        w_sb[kc][:, :, jc * P:(jc + 1) * P],
                        prev[kc],
                        start=(kc == 0),
                        stop=(kc == KC - 1),
                        perf_mode=DR,
                        skip_group_check=True,
                    )
                aeA = aep.tile([P, 2, BS], fp8, tag="aeA", name=f"aeA_{t}")
                aeB = aep.tile([P, 2, BS], fp8, tag="aeB", name=f"aeB_{t}")
                nc.vector.tensor_mul(aeA[:], psA[:, :, 0:BS],
                                     eobs_sb[k][:, toff, 0:2, :])
                nc.vector.tensor_mul(aeB[:], psB[:, :, 0:BS],
                                     eobs_sb[k][:, toff, 2:4, :])
                prev = [aeA[:], aeB[:]]

            # Final: s[b] = sum_z aD_255[z, b] via plain fp8 ones-matmuls.
            psf = psp.tile([1, BS], f32, tag="psA", name="ps_fin")
            for kc in range(KC):
                for pair in range(2):
                    nc.tensor.matmul(psf[:], ones_sb[:],
                                     prev[kc][:, pair, :],
                                     start=(kc == 0 and pair == 0),
                                     stop=(kc == KC - 1 and pair == 1))
            lg = finp.tile([1, BS], f32, name="lg")
            nc.scalar.activation(lg[:], psf[:],
                                 mybir.ActivationFunctionType.Ln)
            nc.sync.dma_start(out=out_d[:], in_=lg[:])

    nc.compile()
    _NC_CACHE["nc"] = nc
    return nc


def _log_softmax64(x, axis):
    x = np.asarray(x, np.float64)
    m = x.max(axis=axis, keepdims=True)
    return x - m - np.log(np.exp(x - m).sum(axis=axis, keepdims=True))


def host_prep(input_ids, T, pi, emit):
    """Numpy prep: normalize params, gather per-step emissions, shard."""
    ids = np.asarray(input_ids).astype(np.int64)
    T_log = _log_softmax64(T, 0)
    pi_log = _log_softmax64(pi, 0)
    emit_log = _log_softmax64(emit, 0)
    W = np.exp(T_log).T  # [i, j] = p(j|i)
    obs = emit_log[ids]  # [256, 64, 512]
    eobs = np.exp(obs[1:] + SHIFT)  # [255, 64, 512]
    ae0 = np.exp(obs[0] + pi_log[None, :])  # [64, 512]

    # rank-1 growth predictor: stationary distribution of W^T
    v = np.ones(Z) / Z
    M = W.T
    for _ in range(50):
        v = M @ v
        v /= v.sum()
    sigma = np.einsum('j,tbj->tb', v, eobs)  # [255, 64]

    e4 = ml_dtypes.float8_e4m3
    bf = ml_dtypes.bfloat16
    # w_pack[kc, p, pair, j] = W[kc*256 + pair*128 + p, j] * SW
    w_pack = np.ascontiguousarray(
        (W * SW).reshape(KC, 2, P, Z).transpose(0, 2, 1, 3).astype(e4))

    a0mean = ae0.mean(axis=1)  # [64]
    a0 = (ae0 / a0mean[:, None] * ASC)  # [64, 512]
    # logC[b]: accumulated host-side log scale
    logC = np.log(a0mean) - np.log(ASC) + np.log(sigma).sum(axis=0)  # [64]

    eobs_s = eobs / (sigma[:, :, None] * SW)  # [255, 64, 512]

    in_maps = []
    for c in range(NCORES):
        bsl = slice(c * BS, (c + 1) * BS)
        e = eobs_s[:, bsl, :].reshape(SEQ - 1, BS, ZC, P)
        e = np.ascontiguousarray(e.transpose(3, 0, 2, 1).astype(bf))
        a = a0[bsl, :].reshape(BS, ZC, P)
        a = np.ascontiguousarray(a.transpose(2, 1, 0).astype(e4))
        in_maps.append({"w": w_pack, "eobs": e, "ae0": a})
    return in_maps, logC


def kernel(input_ids, T, pi, emit, _trace=False):
    from concourse.bass_utils import run_bass_kernel_spmd

    nc = _build_nc()
    in_maps, logC = host_prep(input_ids, T, pi, emit)
    r = run_bass_kernel_spmd(nc, in_maps, core_ids=list(range(NCORES)),
                             trace=_trace)
    lg = np.concatenate([r.results[c]["out"][0] for c in range(NCORES)])
    if _trace:
        kernel.last_results = r
    out = -(lg.astype(np.float64) + logC - (SEQ - 1) * SHIFT)
    return out.astype(np.float32)
